# revision 21
# baseline (speedup 1.0000x reference)
"""Self-contained Trainium2 Bass kernel for MultiHeadAttention.

Problem: B=2, S=2048, D=1024, H=16, hd=64, causal-ish mask with the
reference's masked_fill(mask==0, -1e-09) quirk (masked scores ~ 0.0, so
exp(masked) == 1.0 in fp32 and every key position participates in the
softmax denominator).

Sharding: 8 cores = 2 batches x 4 head-groups (4 heads per core).
Each core computes a partial [S, D] output (its 4 heads pushed through
the O-projection); the host sums the 4 partials per batch and adds bo.
"""

import numpy as np
import ml_dtypes

import concourse.bass as bass
import concourse.bacc as bacc
import concourse.tile as tile
import concourse.mybir as mybir
from concourse.bass_utils import run_bass_kernel_spmd
from concourse.masks import make_identity

BF16 = mybir.dt.bfloat16
F32 = mybir.dt.float32
NPBF16 = ml_dtypes.bfloat16

B = 2
S = 2048
D = 1024
H = 16
HD = 64
NCORES = 8
HPC = 4            # heads per core
NPAIRS = 2         # head pairs per core (row-packed in PE)
NQ = S // 128      # 16 query/key blocks of 128
QCH = 512          # sq chunk width
NCH = S // QCH     # 4 chunks
KT = D // 128      # 8 contraction tiles for projections


def _emit(tc: tile.TileContext, io: dict):
    nc = tc.nc
    ctx_pools = []

    persist = tc.alloc_tile_pool(name="persist", bufs=1)
    ctx_pools.append(persist)

    # ---- constants ----
    ident = persist.tile([128, 128], BF16, name="ident")
    make_identity(nc, ident)
    ones128 = persist.tile([128, 128], BF16, name="ones128")
    nc.gpsimd.memset(ones128, 1.0)
    # maskinv[p, y] = 1 where y < p (strictly-lower free index == masked
    # region of a transposed diagonal block), else 0.
    maskinv = persist.tile([128, 128], mybir.dt.int8, name="maskinv")
    nc.gpsimd.memset(maskinv, 0.0)
    nc.gpsimd.affine_select(
        out=maskinv, in_=maskinv,
        compare_op=mybir.AluOpType.is_ge,
        fill=1.0, base=0,
        pattern=[[1, 128]], channel_multiplier=-1,
    )

    # ---- persistent SBUF arrays ----
    qt = persist.tile([128, NPAIRS, S], BF16, name="qt")      # [2h-dims, pair, s]
    # kT stored zero-padded per head: even heads occupy partitions 0-63
    # (64-127 zero), odd heads occupy 64-127 (0-63 zero), so the scores
    # matmul is a plain K=128 base-0 matmul against the pair-stacked qt.
    ktz = persist.tile([128, HPC, S], BF16, name="ktz")
    v2 = persist.tile([128, HPC, NQ, 65], BF16, name="v2")    # V'' with ones col
    fs = persist.tile([128, HPC, NQ, 65], BF16, name="fs")    # folded suffix sums
    att = persist.tile([128, NPAIRS, S], BF16, name="att")    # attn^T per pair

    # input stages
    qts = persist.tile([128, KT, S], BF16, name="qts")        # Q^T tiles
    kts = persist.tile([128, KT, S], BF16, name="kts")
    vts = persist.tile([128, KT, S], BF16, name="vts")
    wqt = persist.tile([128, KT, 256], BF16, name="wqt")
    wkt = persist.tile([128, KT, 256], BF16, name="wkt")
    wvt = persist.tile([128, KT, 256], BF16, name="wvt")
    wot = persist.tile([128, NPAIRS, D], BF16, name="wot")
    bq_sb = persist.tile([1, 256], BF16, name="bq_sb")
    bk_sb = persist.tile([1, 256], BF16, name="bk_sb")
    bv_sb = persist.tile([1, 256], BF16, name="bv_sb")
    onesrow = persist.tile([1, 512], BF16, name="onesrow")
    nc.gpsimd.memset(onesrow, 1.0)

    dma = nc.default_dma_engine
    for t in range(KT):
        dma.dma_start(qts[:, t, :], io["QT"][t * 128:(t + 1) * 128, :])
        dma.dma_start(kts[:, t, :], io["KT"][t * 128:(t + 1) * 128, :])
        dma.dma_start(vts[:, t, :], io["VT"][t * 128:(t + 1) * 128, :])
        dma.dma_start(wqt[:, t, :], io["WqT"][t * 128:(t + 1) * 128, :])
        dma.dma_start(wkt[:, t, :], io["WkT"][t * 128:(t + 1) * 128, :])
        dma.dma_start(wvt[:, t, :], io["WvT"][t * 128:(t + 1) * 128, :])
    for p in range(NPAIRS):
        dma.dma_start(wot[:, p, :], io["WoT"][p * 128:(p + 1) * 128, :])
    dma.dma_start(bq_sb, io["bq"])
    dma.dma_start(bk_sb, io["bk"])
    dma.dma_start(bv_sb, io["bv"])

    # ================= Phase A: projections =================
    for h in range(HPC):  # zero the unused half of each ktz head
        half = slice(64, 128) if h % 2 == 0 else slice(0, 64)
        nc.vector.memset(ktz[half, h, :], 0.0)
    pa = tc.alloc_tile_pool(name="pa_psum", bufs=4, space="PSUM")
    for p in range(NPAIRS):
        for c in range(NCH):
            sq = slice(c * QCH, (c + 1) * QCH)
            ps_q = pa.tile([128, QCH], F32, tag="ps_q")
            ps_k = pa.tile([128, QCH], F32, tag="ps_q")
            for t in range(KT):
                nc.tensor.matmul(ps_q, wqt[:, t, p * 128:(p + 1) * 128],
                                 qts[:, t, sq], start=(t == 0), stop=False)
            nc.tensor.matmul(ps_q, bq_sb[0:1, p * 128:(p + 1) * 128],
                             onesrow, start=False, stop=True)  # + bq rank-1
            for t in range(KT):
                nc.tensor.matmul(ps_k, wkt[:, t, p * 128:(p + 1) * 128],
                                 kts[:, t, sq], start=(t == 0), stop=False)
            nc.tensor.matmul(ps_k, bk_sb[0:1, p * 128:(p + 1) * 128],
                             onesrow, start=False, stop=True)  # + bk rank-1
            nc.scalar.copy(qt[:, p, sq], ps_q)
            nc.scalar.copy(ktz[0:64, 2 * p, sq], ps_k[0:64, :])
            nc.scalar.copy(ktz[64:128, 2 * p + 1, sq], ps_k[64:128, :])
    # V projection: natural layout [s, 4*64]
    for st in range(NQ):
        ps_v = pa.tile([128, 256], F32, tag="ps_v")
        for t in range(KT):
            nc.tensor.matmul(ps_v, vts[:, t, st * 128:(st + 1) * 128],
                             wvt[:, t, :], start=(t == 0), stop=False)
        nc.tensor.matmul(ps_v, ones128[0:1, :], bv_sb,
                         start=False, stop=True)  # rank-1 bias add
        for h in range(HPC):
            nc.vector.tensor_copy(v2[:, h, st, 0:64], ps_v[:, h * 64:(h + 1) * 64])
    nc.gpsimd.memset(v2[:, :, :, 64:65], 1.0)  # ones column
    pa.release()

    # folded suffixes: fs[:, h, q, :] = sum_{kj > q} v2[:, h, kj, :]
    nc.vector.memset(fs[:, :, NQ - 1, :], 0.0)
    for h in range(HPC):
        for q in range(NQ - 2, -1, -1):
            nc.vector.tensor_add(fs[:, h, q, :], fs[:, h, q + 1, :],
                                 v2[:, h, q + 1, :])

    # ================= Phase B: attention =================
    pb_s = tc.alloc_tile_pool(name="pb_scores", bufs=3, space="PSUM")
    pb_a = tc.alloc_tile_pool(name="pb_attnu", bufs=3, space="PSUM")
    pb_t = tc.alloc_tile_pool(name="pb_trans", bufs=2, space="PSUM")
    pb_e = tc.alloc_tile_pool(name="pb_exp", bufs=4)
    pb_r = tc.alloc_tile_pool(name="pb_recip", bufs=4)

    for p in range(NPAIRS):
        for c in range(NCH):
            au = [pb_a.tile([128, 4, 65], F32, tag="au", name=f"au{p}_{c}_{hl}")
                  for hl in range(2)]
            for kj in range(4 * c + 4):
                c0 = max(kj - 4 * c, 0) * 128   # first valid col in chunk
                ncols = QCH - c0
                for hl in range(2):
                    h = 2 * p + hl
                    sps = pb_s.tile([128, ncols], F32, tag="sps",
                                    name=f"sps{p}_{c}_{kj}_{hl}")
                    nc.tensor.matmul(
                        sps,
                        ktz[:, h, kj * 128:(kj + 1) * 128],
                        qt[:, p, c * QCH + c0:(c + 1) * QCH],
                        start=True, stop=True)
                    ext = pb_e.tile([128, ncols], BF16, tag="ext",
                                    name=f"ext{p}_{c}_{kj}_{hl}")
                    nc.scalar.activation(ext, sps,
                                         mybir.ActivationFunctionType.Exp,
                                         scale=0.125)
                    if kj >= 4 * c:  # diagonal block: force masked exp to 1.0
                        nc.vector.copy_predicated(ext[:, 0:128], maskinv, ones128)
                    for ql in range(4):
                        qi = 4 * c + ql
                        if qi < kj:
                            continue
                        # one accumulation group per bank: start only on the
                        # very first matmul into this au tile, stop on the last
                        nc.tensor.matmul(
                            au[hl][:, ql, :],
                            ext[:, ql * 128 - c0:(ql + 1) * 128 - c0],
                            v2[:, h, kj, :],
                            start=(kj == 0 and ql == 0), stop=False)
            # finalize the 4 query blocks of this chunk: first close each
            # bank's accumulation group (FS rank-1 adds), then read it out
            for hl in range(2):
                h = 2 * p + hl
                last_fs_ql = 2 if c == NCH - 1 else 3  # qi=15 has no FS matmul
                for ql in range(4):
                    qi = 4 * c + ql
                    if qi < NQ - 1:
                        nc.tensor.matmul(au[hl][:, ql, :], ones128,
                                         fs[:, h, qi, :], start=False,
                                         stop=(ql == last_fs_ql))
                for ql in range(4):
                    qi = 4 * c + ql
                    rec = pb_r.tile([128, 1], F32, tag="rec",
                                    name=f"rec{p}_{c}_{hl}_{ql}")
                    nc.vector.reciprocal(rec, au[hl][:, ql, 64:65])
                    ans = pb_r.tile([128, 64], BF16, tag="ans",
                                    name=f"ans{p}_{c}_{hl}_{ql}")
                    nc.vector.tensor_scalar_mul(ans, au[hl][:, ql, 0:64], rec)
                    tr = pb_t.tile([64, 128], BF16, tag="tr",
                                   name=f"tr{p}_{c}_{hl}_{ql}")
                    nc.tensor.transpose(tr, ans, ident)
                    nc.vector.tensor_copy(
                        att[hl * 64:(hl + 1) * 64, p, qi * 128:(qi + 1) * 128], tr)
    pb_r.release()
    pb_e.release()
    pb_t.release()
    pb_a.release()
    pb_s.release()

    # ================= Phase C: output projection =================
    pc = tc.alloc_tile_pool(name="pc_psum", bufs=3, space="PSUM")
    pc_o = tc.alloc_tile_pool(name="pc_out", bufs=3)
    for st in range(NQ):
        for dc in range(2):
            ps_o = pc.tile([128, 512], F32, tag="ps_o", name=f"ps_o{st}_{dc}")
            for p in range(NPAIRS):
                # K=128 contraction = both heads of the pair stacked
                nc.tensor.matmul(
                    ps_o,
                    att[:, p, st * 128:(st + 1) * 128],
                    wot[:, p, dc * 512:(dc + 1) * 512],
                    start=(p == 0), stop=(p == NPAIRS - 1))
            ob = pc_o.tile([128, 512], F32, tag="ob", name=f"ob{st}_{dc}")
            nc.vector.tensor_copy(ob, ps_o)
            dma.dma_start(io["out"][st * 128:(st + 1) * 128,
                                    dc * 512:(dc + 1) * 512], ob)
    pc_o.release()
    pc.release()
    persist.release()


_CACHED = None


def _build():
    global _CACHED
    if _CACHED is not None:
        return _CACHED
    nc = bacc.Bacc("TRN2", target_bir_lowering=False, debug=False)
    io = {
        "QT": nc.dram_tensor("QT", [D, S], BF16, kind="ExternalInput").ap(),
        "KT": nc.dram_tensor("KT", [D, S], BF16, kind="ExternalInput").ap(),
        "VT": nc.dram_tensor("VT", [D, S], BF16, kind="ExternalInput").ap(),
        "WqT": nc.dram_tensor("WqT", [D, 256], BF16, kind="ExternalInput").ap(),
        "WkT": nc.dram_tensor("WkT", [D, 256], BF16, kind="ExternalInput").ap(),
        "WvT": nc.dram_tensor("WvT", [D, 256], BF16, kind="ExternalInput").ap(),
        "WoT": nc.dram_tensor("WoT", [256, D], BF16, kind="ExternalInput").ap(),
        "bq": nc.dram_tensor("bq", [1, 256], BF16, kind="ExternalInput").ap(),
        "bk": nc.dram_tensor("bk", [1, 256], BF16, kind="ExternalInput").ap(),
        "bv": nc.dram_tensor("bv", [1, 256], BF16, kind="ExternalInput").ap(),
        "out": nc.dram_tensor("out", [S, D], F32, kind="ExternalOutput").ap(),
    }
    with tile.TileContext(nc) as tc:
        _emit(tc, io)
    nc.compile()
    _CACHED = (nc, io)
    return _CACHED


def make_in_maps(Q, K, V, Wq, bq, Wk, bk, Wv, bv, Wo):
    """Build the 8 per-core input dicts (host-side sharding)."""
    Q = np.asarray(Q, np.float32)
    K = np.asarray(K, np.float32)
    V = np.asarray(V, np.float32)
    qt = [np.ascontiguousarray(Q[b].T).astype(NPBF16) for b in range(B)]
    kt = [np.ascontiguousarray(K[b].T).astype(NPBF16) for b in range(B)]
    vt = [np.ascontiguousarray(V[b].T).astype(NPBF16) for b in range(B)]
    in_maps = []
    for core in range(NCORES):
        b, g = divmod(core, 4)
        rows = slice(g * 256, (g + 1) * 256)
        in_maps.append({
            "QT": qt[b], "KT": kt[b], "VT": vt[b],
            "WqT": np.ascontiguousarray(np.asarray(Wq, np.float32)[rows].T).astype(NPBF16),
            "WkT": np.ascontiguousarray(np.asarray(Wk, np.float32)[rows].T).astype(NPBF16),
            "WvT": np.ascontiguousarray(np.asarray(Wv, np.float32)[rows].T).astype(NPBF16),
            "WoT": np.ascontiguousarray(np.asarray(Wo, np.float32)[:, rows].T).astype(NPBF16),
            "bq": np.asarray(bq, np.float32)[rows].reshape(1, 256).astype(NPBF16),
            "bk": np.asarray(bk, np.float32)[rows].reshape(1, 256).astype(NPBF16),
            "bv": np.asarray(bv, np.float32)[rows].reshape(1, 256).astype(NPBF16),
        })
    return in_maps


def kernel(Q, K, V, mask, Wq, bq, Wk, bk, Wv, bv, Wo, bo, _results_hook=None):
    nc, _io = _build()
    in_maps = make_in_maps(Q, K, V, Wq, bq, Wk, bk, Wv, bv, Wo)
    res = run_bass_kernel_spmd(nc, in_maps, core_ids=list(range(NCORES)))
    if _results_hook is not None:
        _results_hook(res)
    out = np.zeros((B, S, D), np.float32)
    for core in range(NCORES):
        out[core // 4] += res.results[core]["out"]
    out += np.asarray(bo, np.float32)
    return out
